# revision 1
# baseline (speedup 1.0000x reference)
"""BNAF forward + log-det on 8 TRN2 NeuronCores (self-contained).

Sharding: data-parallel over batch (128 rows/core), params replicated.
Host does layout-only prep (transpose / gather / structural masking / constant
fills); all arithmetic on input values happens on device.

Math restructuring (validated vs reference in numpy, rel err ~4e-7):
  masked weight: w = raw-lower-blocks + exp(diag-blocks);  out = (x @ w.T)*s + b
  with s[r] = exp(dw[r] - 0.5*ln(wsn[r])), wsn = row sums of w^2.
  Jacobian log-det chain in linear domain:
    exp(log|dtanh|) = 1 - tanh^2
    E0 = wn0_diag*(1-h0^2); E1 = blockdiag(wn1)@E0*(1-h1^2); E2 = blockdiag(wn2)@E1
    ldj = sum_d [ln(1 + e^gate * E2) - ln(1 + e^gate)] (flows 0,1); sum_d ln E2 (flow 2)
"""
import numpy as np

DIM, HID, B = 64, 32, 1024
NCORES = 8
BC = B // NCORES
DH = DIM * HID             # 2048
NK = DH // 128             # 16
LOG_2PI = float(np.log(2.0 * np.pi))
NEG = -1e30
MAIN_DT = 'float32r'       # 'float32' or 'float32r'

_CACHE = {}


# ---------------------------------------------------------------- host prep
def _host_prep(inputs):
    fl = {}
    r = np.arange(DH)
    blk = r // HID
    c64 = np.arange(DIM)
    for f in range(3):
        W0 = np.asarray(inputs[f'W{f}_0'])
        W1 = np.asarray(inputs[f'W{f}_1'])
        W2 = np.asarray(inputs[f'W{f}_2'])

        W0T = np.ascontiguousarray(W0.T)                      # (64, 2048)
        keep = c64[:, None] < blk[None, :]
        diag = c64[:, None] == blk[None, :]
        fl[f'w0raw{f}'] = np.where(keep, W0T, 0.0).astype(np.float32)
        fl[f'w0exp{f}'] = np.where(diag, W0T, NEG).astype(np.float32)

        W1T = np.ascontiguousarray(W1.T)                      # (2048, 2048)
        keep1 = (blk[:, None] < blk[None, :]) & (
            (r[:, None] // 128) != (r[None, :] // 128))
        fl[f'w1raw{f}'] = np.where(keep1, W1T, 0.0).astype(np.float32)
        d_raw = np.zeros((128, DH), np.float32)
        d_exp = np.full((128, DH), NEG, np.float32)
        i = np.arange(128)
        low = (i[:, None] // HID) < (i[None, :] // HID)
        dia = (i[:, None] // HID) == (i[None, :] // HID)
        for k in range(NK):
            t = W1T[128 * k:128 * k + 128, 128 * k:128 * k + 128]
            d_raw[:, 128 * k:128 * k + 128] = np.where(low, t, 0.0)
            d_exp[:, 128 * k:128 * k + 128] = np.where(dia, t, NEG)
        fl[f'w1draw{f}'] = d_raw
        fl[f'w1dexp{f}'] = d_exp

        W2T = np.ascontiguousarray(W2.T)                      # (2048, 64)
        keep2 = c64[None, :] > blk[:, None]
        diag2 = c64[None, :] == blk[:, None]
        fl[f'w2raw{f}'] = np.where(keep2, W2T, 0.0).astype(np.float32)
        w2e = np.where(diag2, W2T, NEG).astype(np.float32)    # (2048, 64)
        fl[f'w2exp{f}'] = np.ascontiguousarray(
            w2e.reshape(NK, 128, DIM).transpose(1, 0, 2).reshape(128, NK * DIM))

        fl[f'dw0c{f}'] = np.ascontiguousarray(
            np.asarray(inputs[f'dw{f}_0'])[:, 0].reshape(NK, 128).T)
        fl[f'dw1c{f}'] = np.ascontiguousarray(
            np.asarray(inputs[f'dw{f}_1'])[:, 0].reshape(NK, 128).T)
        fl[f'dw2c{f}'] = np.asarray(inputs[f'dw{f}_2']).astype(np.float32)
        fl[f'b0c{f}'] = np.ascontiguousarray(
            np.asarray(inputs[f'b{f}_0']).reshape(NK, 128).T)
        fl[f'b1c{f}'] = np.ascontiguousarray(
            np.asarray(inputs[f'b{f}_1']).reshape(NK, 128).T)
        fl[f'b2c{f}'] = np.asarray(inputs[f'b{f}_2']).reshape(DIM, 1).astype(np.float32)
        fl[f'g0c{f}'] = np.ascontiguousarray(
            W0[np.arange(DH), blk].reshape(NK, 128).T.astype(np.float32))

    fl['gatec0'] = np.full((DIM, 1), float(np.asarray(inputs['gate0'])[0]), np.float32)
    fl['gatec1'] = np.full((DIM, 1), float(np.asarray(inputs['gate1'])[0]), np.float32)
    fl['flip64'] = np.eye(DIM, dtype=np.float32)[:, ::-1].copy()
    fl['ones64'] = np.ones((DIM, 1), np.float32)
    fl['onescol'] = np.ones((128, 1), np.float32)
    return fl


# ------------------------------------------------- walrus sync-wait splitter
def _split_sync_waits(nc, max_waits=1):
    import concourse.mybir as mybir
    for func in nc.m.functions:
        for blkb in func.blocks:
            insts = list(blkb.instructions)
            out = []
            changed = False
            for inst in insts:
                si = inst.sync_info
                if si is not None and len(si.on_wait) > max_waits:
                    waits = list(si.on_wait)
                    keep, pre = waits[-max_waits:], waits[:-max_waits]
                    chunks = [pre[i:i + max_waits] for i in range(0, len(pre), max_waits)]
                    for ci, chunk in enumerate(chunks):
                        nop = mybir.InstNoOp(name=f"{inst.name}.w{ci}", ins=[], outs=[])
                        nop.engine = inst.engine
                        nop.sync_info = mybir.SyncInfo(on_wait=chunk, on_update=[])
                        out.append(nop)
                    inst.sync_info = mybir.SyncInfo(
                        on_wait=keep, on_update=list(si.on_update))
                    changed = True
                out.append(inst)
            if changed:
                try:
                    blkb.instructions = out
                except Exception:
                    while len(blkb.instructions):
                        blkb.remove_instruction(blkb.instructions[-1])
                    for i2 in out:
                        blkb.add_instruction(i2)


# ---------------------------------------------------------------- bass build
def _build(main_dt_name, repeat=1):
    import concourse.bass as bass
    import concourse.mybir as mybir
    import concourse.tile as tile

    f32 = mybir.dt.float32
    bf16 = mybir.dt.bfloat16
    mdt = getattr(mybir.dt, main_dt_name)
    cast = (main_dt_name != 'float32')
    AO = mybir.AluOpType
    AF = mybir.ActivationFunctionType

    nc = bass.Bass()
    P = {}

    def dram(name, shape):
        P[name] = nc.declare_dram_parameter(name, list(shape), f32, isOutput=False)

    dram('xT', (DIM, BC))
    for f in range(3):
        dram(f'w0raw{f}', (DIM, DH)); dram(f'w0exp{f}', (DIM, DH))
        dram(f'w1raw{f}', (DH, DH))
        dram(f'w1draw{f}', (128, DH)); dram(f'w1dexp{f}', (128, DH))
        dram(f'w2raw{f}', (DH, DIM)); dram(f'w2exp{f}', (128, NK * DIM))
        dram(f'dw0c{f}', (128, NK)); dram(f'dw1c{f}', (128, NK)); dram(f'dw2c{f}', (DIM, 1))
        dram(f'b0c{f}', (128, NK)); dram(f'b1c{f}', (128, NK)); dram(f'b2c{f}', (DIM, 1))
        dram(f'g0c{f}', (128, NK))
    dram('gatec0', (DIM, 1)); dram('gatec1', (DIM, 1))
    dram('flip64', (DIM, DIM)); dram('ones64', (DIM, 1)); dram('onescol', (128, 1))
    out = nc.declare_dram_parameter('out', [1, BC], f32, isOutput=True)
    DBG = {}
    import os as _os
    if _os.environ.get('KDEBUG'):
        for nm, shp in [('d_s0', (128, NK)), ('d_s1', (128, NK)), ('d_s2', (DIM, 1)),
                        ('d_h0', (128, BC)), ('d_E0', (128, BC)), ('d_h1', (128, BC)),
                        ('d_E1', (128, BC)), ('d_E2', (DIM, BC)), ('d_z2', (DIM, BC)),
                        ('d_x1', (DIM, BC))]:
            DBG[nm] = nc.declare_dram_parameter(nm, list(shp), f32, isOutput=True)

    with tile.TileContext(nc) as tc:
        with tc.tile_pool(name='const', bufs=1) as cpool, \
             tc.tile_pool(name='wchunk', bufs=1) as wpool, \
             tc.tile_pool(name='acts', bufs=1) as apool, \
             tc.tile_pool(name='sq', bufs=1) as sqpool, \
             tc.tile_pool(name='small', bufs=2) as spool, \
             tc.tile_pool(name='psz', bufs=2, space='PSUM') as pp, \
             tc.tile_pool(name='pse', bufs=2, space='PSUM') as pp1, \
             tc.tile_pool(name='psw', bufs=2, space='PSUM') as ppw:

            def mload(pool, tag, shape, dt, srcap, bufs=None):
                t = pool.tile(list(shape), dt, name=tag, tag=tag, bufs=bufs)
                if dt != f32:
                    nc.gpsimd.dma_start(out=t[:], in_=srcap)
                else:
                    nc.sync.dma_start(out=t[:], in_=srcap)
                return t

            onescol_bf = mload(cpool, 'onescol_bf', (128, 1), bf16, P['onescol'][:])
            onescol_f = mload(cpool, 'onescol_f', (128, 1), f32, P['onescol'][:])
            ones64 = mload(cpool, 'ones64', (DIM, 1), f32, P['ones64'][:])
            flip64 = mload(cpool, 'flip64', (DIM, DIM), mdt, P['flip64'][:])
            xT = mload(cpool, 'xT', (DIM, BC), mdt, P['xT'][:])

            for _rep in range(repeat):
              if _rep:
                xT = mload(cpool, 'xT', (DIM, BC), mdt, P['xT'][:])
              acc = cpool.tile([DIM, BC], f32, name='acc', tag='acc')
              nc.vector.memset(acc[:], 0.0)
              E2s = [cpool.tile([DIM, BC], f32, name=f'E2_{f}', tag=f'E2_{f}') for f in range(3)]
              egs = [cpool.tile([DIM, 1], f32, name=f'eg_{f}', tag=f'eg_{f}') for f in range(2)]

              for f in range(3):
                    # ---- loads
                    w0raw = mload(apool, 'w0raw', (DIM, DH), mdt, P[f'w0raw{f}'][:])
                    w0exp_in = mload(apool, 'expin', (DIM, DH), f32, P[f'w0exp{f}'][:])
                    w1chunks = [
                        mload(wpool, f'w1c{k}', (128, DH - 128 * k), mdt,
                              P[f'w1raw{f}'][128 * k:128 * k + 128, 128 * k:],
                              bufs=(2 if k >= 7 else None))
                        for k in range(NK)]
                    w1draw = mload(apool, 'w1draw', (128, DH), mdt, P[f'w1draw{f}'][:])
                    w1dexp_in = mload(apool, 'expin', (128, DH), f32, P[f'w1dexp{f}'][:])
                    w2chunks = [
                        mload(wpool, f'w2c{k}', (128, DIM), mdt,
                              P[f'w2raw{f}'][128 * k:128 * k + 128, :])
                        for k in range(NK)]
                    w2exp_in = mload(apool, 'w2exp_in', (128, NK * DIM), f32, P[f'w2exp{f}'][:])
                    dw0 = mload(spool, 'dw0', (128, NK), f32, P[f'dw0c{f}'][:])
                    dw1 = mload(spool, 'dw1', (128, NK), f32, P[f'dw1c{f}'][:])
                    dw2 = mload(spool, 'dw2', (DIM, 1), f32, P[f'dw2c{f}'][:])
                    b0 = mload(spool, 'b0', (128, NK), f32, P[f'b0c{f}'][:])
                    b1 = mload(spool, 'b1', (128, NK), f32, P[f'b1c{f}'][:])
                    b2 = mload(spool, 'b2', (DIM, 1), f32, P[f'b2c{f}'][:])
                    g0c = mload(spool, 'g0c', (128, NK), f32, P[f'g0c{f}'][:])

                    # ---- device exps of filled diag tensors
                    w0exp = apool.tile([DIM, DH], mdt, name='w0exp_m', tag='w0exp_m')
                    nc.scalar.activation(w0exp[:], w0exp_in[:], AF.Exp)
                    w1dexp = apool.tile([128, DH], mdt, name='w1dexp_m', tag='w1dexp_m')
                    nc.scalar.activation(w1dexp[:], w1dexp_in[:], AF.Exp)
                    w1dexp_bf = apool.tile([128, DH], bf16, name='w1dexp_bf', tag='w1dexp_bf')
                    nc.scalar.activation(w1dexp_bf[:], w1dexp_in[:], AF.Exp)
                    w2expm = apool.tile([128, NK * DIM], mdt, name='w2exp_m', tag='w2exp_m')
                    nc.scalar.activation(w2expm[:], w2exp_in[:], AF.Exp)
                    w2exp_bf = apool.tile([128, NK * DIM], bf16, name='w2exp_bf', tag='w2exp_bf')
                    nc.scalar.activation(w2exp_bf[:], w2exp_in[:], AF.Exp)

                    # ---- wsn: squares (bf16) then N=1 matmul column sums
                    sq0 = sqpool.tile([128, DH], bf16, name='sq0', tag='sq0')
                    nc.scalar.square(sq0[:DIM, :], w0raw[:])
                    nc.scalar.activation(sq0[DIM:, :], w0exp_in[:], AF.Exp, scale=2.0)
                    wsn0p = ppw.tile([128, NK], f32, name='wsnp', tag='wsnp')
                    for m in range(NK):
                        nc.tensor.matmul(wsn0p[:, m:m + 1], sq0[:, 128 * m:128 * m + 128],
                                         onescol_bf[:], start=True, stop=True)
                    s0 = _scol(nc, spool, AO, AF, wsn0p, dw0, (128, NK), 's0')

                    # wsn1: per-chunk transient squares; single start/stop psum groups
                    # per column per chunk (no interleaved accumulation within a bank),
                    # accumulated in SBUF via DVE adds.
                    wsn1_sb = spool.tile([128, NK], f32, name='wsn1sb', tag='wsn1sb')
                    nc.vector.memset(wsn1_sb[:], 0.0)
                    for k in range(NK - 1):
                        s = sqpool.tile([128, DH - 128 * (k + 1)], bf16,
                                        name='sq1t', tag='sq1t')
                        eng = (nc.scalar, nc.vector, nc.gpsimd)[k % 3]
                        if eng is nc.scalar:
                            nc.scalar.square(s[:], w1chunks[k][:, 128:])
                        else:
                            eng.tensor_tensor(s[:], w1chunks[k][:, 128:],
                                              w1chunks[k][:, 128:], AO.mult)
                        psk = ppw.tile([128, NK], f32, name='wsnp', tag='wsnp')
                        for m in range(k + 1, NK):
                            nc.tensor.matmul(
                                psk[:, m:m + 1],
                                s[:, 128 * (m - k - 1):128 * (m - k - 1) + 128],
                                onescol_bf[:], start=True, stop=True)
                        nc.vector.tensor_tensor(wsn1_sb[:, k + 1:], wsn1_sb[:, k + 1:],
                                                psk[:, k + 1:], AO.add)
                    sqd1a = sqpool.tile([128, DH], bf16, name='sqd1a', tag='sqd1a')
                    nc.vector.tensor_tensor(sqd1a[:], w1draw[:], w1draw[:], AO.mult)
                    sqd1b = sqpool.tile([128, DH], bf16, name='sqd1b', tag='sqd1b')
                    nc.scalar.activation(sqd1b[:], w1dexp_in[:], AF.Exp, scale=2.0)
                    psd = ppw.tile([128, NK], f32, name='wsnp', tag='wsnp')
                    for m in range(NK):
                        nc.tensor.matmul(psd[:, m:m + 1], sqd1a[:, 128 * m:128 * m + 128],
                                         onescol_bf[:], start=True, stop=False)
                        nc.tensor.matmul(psd[:, m:m + 1], sqd1b[:, 128 * m:128 * m + 128],
                                         onescol_bf[:], start=False, stop=True)
                    nc.vector.tensor_tensor(wsn1_sb[:], wsn1_sb[:], psd[:], AO.add)
                    s1 = _scol(nc, spool, AO, AF, wsn1_sb, dw1, (128, NK), 's1')

                    sq2 = sqpool.tile([128, NK * DIM], bf16, name='sq2', tag='sq2')
                    for k in range(NK):
                        nc.vector.tensor_tensor(sq2[:, k * DIM:(k + 1) * DIM],
                                                w2chunks[k][:], w2chunks[k][:], AO.mult)
                    sq2b = sqpool.tile([128, NK * DIM], bf16, name='sq2b', tag='sq2b')
                    nc.scalar.activation(sq2b[:], w2exp_in[:], AF.Exp, scale=2.0)
                    wsn2p = ppw.tile([DIM, 1], f32, name='wsnp', tag='wsnp')
                    for k in range(NK):
                        nc.tensor.matmul(wsn2p[:], sq2[:, k * DIM:(k + 1) * DIM],
                                         onescol_bf[:], start=(k == 0), stop=False)
                    for k in range(NK):
                        nc.tensor.matmul(wsn2p[:], sq2b[:, k * DIM:(k + 1) * DIM],
                                         onescol_bf[:], start=False, stop=(k == NK - 1))
                    s2 = _scol(nc, spool, AO, AF, wsn2p, dw2, (DIM, 1), 's2')

                    # c0 = s0 * exp(g0raw); and its negation
                    eg0 = spool.tile([128, NK], f32, name='eg0', tag='eg0')
                    nc.scalar.activation(eg0[:], g0c[:], AF.Exp)
                    c0 = spool.tile([128, NK], f32, name='c0', tag='c0')
                    nc.vector.tensor_tensor(c0[:], eg0[:], s0[:], AO.mult)
                    c0n = spool.tile([128, NK], f32, name='c0n', tag='c0n')
                    nc.vector.tensor_scalar(out=c0n[:], in0=c0[:], scalar1=-1.0,
                                            scalar2=None, op0=AO.mult)
                    s1n = spool.tile([128, NK], f32, name='s1n', tag='s1n')
                    nc.vector.tensor_scalar(out=s1n[:], in0=s1[:], scalar1=-1.0,
                                            scalar2=None, op0=AO.mult)

                    # ---- layer 0
                    h0 = [apool.tile([128, BC], mdt, name=f'h0_{m}', tag=f'h0_{m}') for m in range(NK)]
                    E0 = [apool.tile([128, BC], bf16, name=f'E0_{m}', tag=f'E0_{m}') for m in range(NK)]
                    for g in range(4):
                        ps = pp.tile([128, 512], f32, name='zmain', tag='zmain')
                        for j in range(4):
                            m = 4 * g + j
                            sl = ps[:, 128 * j:128 * j + 128]
                            nc.tensor.matmul(sl, w0raw[:, 128 * m:128 * m + 128], xT[:],
                                             start=True, stop=False)
                            nc.tensor.matmul(sl, w0exp[:, 128 * m:128 * m + 128], xT[:],
                                             start=False, stop=True)
                        for j in range(4):
                            m = 4 * g + j
                            sl = ps[:, 128 * j:128 * j + 128]
                            nc.scalar.activation(h0[m][:], sl, AF.Tanh,
                                                 bias=b0[:, m:m + 1], scale=s0[:, m:m + 1])
                            hsq = spool.tile([128, BC], bf16, name='hsq', tag='hsq')
                            nc.vector.tensor_tensor(hsq[:], h0[m][:], h0[m][:], AO.mult)
                            nc.vector.tensor_scalar(out=E0[m][:], in0=hsq[:],
                                                    scalar1=c0n[:, m:m + 1],
                                                    scalar2=c0[:, m:m + 1],
                                                    op0=AO.mult, op1=AO.add)

                    # ---- layer 1 (+ combine)
                    h1 = [apool.tile([128, BC], mdt, name=f'h1_{m}', tag=f'h1_{m}') for m in range(NK)]
                    E1 = [apool.tile([128, BC], bf16, name=f'E1_{m}', tag=f'E1_{m}') for m in range(NK)]
                    for g in range(4):
                        ps = pp.tile([128, 512], f32, name='zmain', tag='zmain')
                        for j in range(4):
                            m = 4 * g + j
                            sl = ps[:, 128 * j:128 * j + 128]
                            for k in range(m):
                                nc.tensor.matmul(
                                    sl, w1chunks[k][:, 128 * (m - k):128 * (m - k) + 128],
                                    h0[k][:], start=(k == 0), stop=False)
                            nc.tensor.matmul(sl, w1draw[:, 128 * m:128 * m + 128], h0[m][:],
                                             start=(m == 0), stop=False)
                            nc.tensor.matmul(sl, w1dexp[:, 128 * m:128 * m + 128], h0[m][:],
                                             start=False, stop=True)
                        psE = pp1.tile([128, 512], f32, name='epath', tag='epath')
                        for j in range(4):
                            m = 4 * g + j
                            nc.tensor.matmul(psE[:, 128 * j:128 * j + 128],
                                             w1dexp_bf[:, 128 * m:128 * m + 128], E0[m][:],
                                             start=True, stop=True)
                        for j in range(4):
                            m = 4 * g + j
                            sl = ps[:, 128 * j:128 * j + 128]
                            nc.scalar.activation(h1[m][:], sl, AF.Tanh,
                                                 bias=b1[:, m:m + 1], scale=s1[:, m:m + 1])
                            hsq = spool.tile([128, BC], bf16, name='hsq', tag='hsq')
                            nc.vector.tensor_tensor(hsq[:], h1[m][:], h1[m][:], AO.mult)
                            ets = spool.tile([128, BC], bf16, name='ets', tag='ets')
                            nc.vector.tensor_scalar(out=ets[:], in0=hsq[:],
                                                    scalar1=s1n[:, m:m + 1],
                                                    scalar2=s1[:, m:m + 1],
                                                    op0=AO.mult, op1=AO.add)
                            nc.vector.tensor_tensor(E1[m][:], psE[:, 128 * j:128 * j + 128],
                                                    ets[:], AO.mult)

                    # ---- layer 2
                    psz2 = pp.tile([DIM, BC], f32, name='zsmall', tag='zsmall')
                    for k in range(NK):
                        nc.tensor.matmul(psz2[:], w2chunks[k][:], h1[k][:],
                                         start=(k == 0), stop=False)
                    for k in range(NK):
                        nc.tensor.matmul(psz2[:], w2expm[:, k * DIM:(k + 1) * DIM], h1[k][:],
                                         start=False, stop=(k == NK - 1))
                    z2 = spool.tile([DIM, BC], f32, name='z2s', tag='z2s')
                    nc.scalar.activation(z2[:], psz2[:], AF.Identity,
                                         bias=b2[:, 0:1], scale=s2[:, 0:1])
                    psE2 = pp1.tile([DIM, BC], f32, name='epath', tag='epath')
                    for k in range(NK):
                        nc.tensor.matmul(psE2[:], w2exp_bf[:, k * DIM:(k + 1) * DIM],
                                         E1[k][:], start=(k == 0), stop=(k == NK - 1))
                    nc.vector.tensor_scalar(out=E2s[f][:], in0=psE2[:],
                                            scalar1=s2[:, 0:1], scalar2=None,
                                            op0=AO.mult)

                    if f == 0 and DBG:
                        def _st(nm, t):
                            tmp = spool.tile(list(t.shape), f32, name='dbg' + nm, tag='dbg' + nm)
                            nc.vector.tensor_copy(tmp[:], t[:])
                            nc.sync.dma_start(out=DBG[nm][:], in_=tmp[:])
                        _st('d_s0', s0); _st('d_s1', s1); _st('d_s2', s2)
                        _st('d_h0', h0[1]); _st('d_E0', E0[1])
                        _st('d_h1', h1[1]); _st('d_E1', E1[1])
                        _st('d_E2', E2s[0]); _st('d_z2', z2)
                    # ---- gate mix / flip or final logp term
                    if f < 2:
                        gc = mload(spool, 'gc', (DIM, 1), f32, P[f'gatec{f}'][:])
                        th = spool.tile([DIM, 1], f32, name='th', tag='th')
                        nc.scalar.activation(th[:], gc[:], AF.Tanh, scale=0.5)
                        sg = spool.tile([DIM, 1], f32, name='sg', tag='sg')
                        nc.vector.tensor_scalar(out=sg[:], in0=th[:], scalar1=0.5,
                                                scalar2=0.5, op0=AO.mult, op1=AO.add)
                        nc.scalar.activation(egs[f][:], gc[:], AF.Exp)
                        ta = spool.tile([DIM, BC], f32, name='ta', tag='ta')
                        nc.vector.tensor_scalar(out=ta[:], in0=z2[:], scalar1=sg[:],
                                                scalar2=None, op0=AO.mult)
                        omsg = spool.tile([DIM, 1], f32, name='omsg', tag='omsg')
                        nc.vector.tensor_scalar(out=omsg[:], in0=sg[:], scalar1=-1.0,
                                                scalar2=1.0, op0=AO.mult, op1=AO.add)
                        tb = spool.tile([DIM, BC], f32, name='tb', tag='tb')
                        nc.vector.tensor_scalar(out=tb[:], in0=xT[:], scalar1=omsg[:],
                                                scalar2=None, op0=AO.mult)
                        xmix = spool.tile([DIM, BC], mdt, name='xmix', tag='xmix')
                        nc.vector.tensor_tensor(xmix[:], ta[:], tb[:], AO.add)
                        psf = pp.tile([DIM, BC], f32, name='zsmall', tag='zsmall')
                        nc.tensor.matmul(psf[:], flip64[:], xmix[:], start=True, stop=True)
                        nc.scalar.activation(xT[:], psf[:], AF.Copy)
                        if f == 0 and DBG:
                            tmpx = spool.tile([DIM, BC], f32, name='dbgx1', tag='dbgx1')
                            nc.vector.tensor_copy(tmpx[:], xT[:])
                            nc.sync.dma_start(out=DBG['d_x1'][:], in_=tmpx[:])
                    else:
                        sqx = spool.tile([DIM, BC], f32, name='sqx', tag='sqx')
                        nc.scalar.square(sqx[:], z2[:])
                        nc.vector.tensor_scalar(out=sqx[:], in0=sqx[:], scalar1=-0.5,
                                                scalar2=-0.5 * LOG_2PI, op0=AO.mult, op1=AO.add)
                        nc.vector.tensor_tensor(acc[:], acc[:], sqx[:], AO.add)

            # ---- ldj tail (ln phase)
              for f in range(2):
                    u = spool.tile([DIM, BC], f32, name='u', tag='u')
                    nc.vector.tensor_scalar(out=u[:], in0=E2s[f][:], scalar1=egs[f][:],
                                            scalar2=1.0, op0=AO.mult, op1=AO.add)
                    lf = spool.tile([DIM, BC], f32, name='lf', tag='lf')
                    nc.scalar.activation(lf[:], u[:], AF.Ln)
                    l1p = spool.tile([DIM, 1], f32, name='l1p', tag='l1p')
                    nc.vector.tensor_scalar(out=l1p[:], in0=egs[f][:], scalar1=1.0,
                                            scalar2=None, op0=AO.add)
                    nc.scalar.activation(l1p[:], l1p[:], AF.Ln)
                    nc.vector.tensor_scalar(out=lf[:], in0=lf[:], scalar1=l1p[:],
                                            scalar2=None, op0=AO.subtract)
                    nc.vector.tensor_tensor(acc[:], acc[:], lf[:], AO.add)
              lf2 = spool.tile([DIM, BC], f32, name='lf2', tag='lf2')
              nc.scalar.activation(lf2[:], E2s[2][:], AF.Ln)
              nc.vector.tensor_tensor(acc[:], acc[:], lf2[:], AO.add)

              psum_out = pp.tile([1, BC], f32, name='zsmall', tag='zsmall')
              nc.tensor.matmul(psum_out[:], ones64[:], acc[:], start=True, stop=True)
              outs = spool.tile([1, BC], f32, name='outs', tag='outs')
              nc.vector.tensor_copy(outs[:], psum_out[:])
              nc.sync.dma_start(out=out[:], in_=outs[:])

    _split_sync_waits(nc)
    return nc


def _scol(nc, spool, AO, AF, wsnp, dwc, shape, tag):
    """s = exp(dw - 0.5*ln(wsn)) as (P, ncols) tile."""
    import concourse.mybir as mybir
    f32 = mybir.dt.float32
    ln = spool.tile(list(shape), f32, name=tag + 'n' + 'ln')
    nc.scalar.activation(ln[:], wsnp[:], AF.Ln)
    t = spool.tile(list(shape), f32, name=tag + 'n' + 't')
    nc.vector.tensor_scalar(out=t[:], in0=ln[:], scalar1=-0.5, scalar2=None, op0=AO.mult)
    nc.vector.tensor_tensor(t[:], t[:], dwc[:], AO.add)
    s = spool.tile(list(shape), f32, name=tag + 'n' + 's')
    nc.scalar.activation(s[:], t[:], AF.Exp)
    return s


# ------------------------------------------------------------------ runner
def _make_runner(nc, n_cores):
    import jax
    from jax.sharding import Mesh, PartitionSpec
    from jax.experimental.shard_map import shard_map
    import concourse.mybir as mybir
    from concourse.bass2jax import (_bass_exec_p, partition_id_tensor,
                                    install_neuronx_cc_hook)
    install_neuronx_cc_hook()
    partition_name = nc.partition_id_tensor.name if nc.partition_id_tensor else None
    in_names, out_names, out_avals = [], [], []
    for alloc in nc.m.functions[0].allocations:
        if not isinstance(alloc, mybir.MemoryLocationSet):
            continue
        name = alloc.memorylocations[0].name
        if alloc.kind == "ExternalInput":
            if name != partition_name:
                in_names.append(name)
        elif alloc.kind == "ExternalOutput":
            out_names.append(name)
            out_avals.append(jax.core.ShapedArray(
                tuple(alloc.tensor_shape), mybir.dt.np(alloc.dtype)))
    n_params = len(in_names)
    all_names = in_names + out_names + ([partition_name] if partition_name else [])

    def _body(*args):
        operands = list(args)
        if partition_name is not None:
            operands.append(partition_id_tensor())
        outs = _bass_exec_p.bind(
            *operands, out_avals=tuple(out_avals), in_names=tuple(all_names),
            out_names=tuple(out_names), lowering_input_output_aliases=(),
            sim_require_finite=False, sim_require_nnan=False, nc=nc)
        return tuple(outs)

    devices = jax.devices()[:n_cores]
    mesh = Mesh(np.asarray(devices), ("core",))
    n_outs = len(out_names)
    in_specs = (PartitionSpec("core"),) * (n_params + n_outs)
    out_specs = (PartitionSpec("core"),) * n_outs
    fn = jax.jit(shard_map(_body, mesh=mesh, in_specs=in_specs,
                           out_specs=out_specs, check_rep=False),
                 keep_unused=True)
    return fn, in_names, out_names, out_avals


def _get_runner():
    key = ('runner', MAIN_DT)
    if key not in _CACHE:
        import sys, os
        d = os.path.dirname(os.path.abspath(__file__))
        if d not in sys.path:
            sys.path.insert(0, d)
        nc = _build(MAIN_DT)
        _CACHE[key] = _make_runner(nc, NCORES)
    return _CACHE[key]


def kernel(**inputs):
    fl = _host_prep(inputs)
    x = np.asarray(inputs['x'])
    fn, in_names, out_names, out_avals = _get_runner()
    in_maps = []
    for c in range(NCORES):
        m = dict(fl)
        m['xT'] = np.ascontiguousarray(x[c * BC:(c + 1) * BC, :].T)
        in_maps.append(m)
    concat_in = [np.concatenate([np.asarray(m[name]) for m in in_maps], axis=0)
                 for name in in_names]
    concat_zeros = [np.zeros((NCORES * a.shape[0], *a.shape[1:]), a.dtype)
                    for a in out_avals]
    outs = fn(*concat_in, *concat_zeros)
    o = np.asarray(outs[0]).reshape(NCORES, BC)
    return o.reshape(B).astype(np.float32)



# revision 8
# speedup vs baseline: 1.8820x; 1.8820x over previous
"""BNAF forward + log-det on 8 TRN2 NeuronCores (self-contained), v2.

Sharding: data-parallel over batch (128 rows/core), params replicated.
Host does layout-only prep (transpose / slice / gather / structural masking /
constant fills / dtype staging to bf16); all arithmetic on input values
happens on device.

Math (validated vs reference):
  masked weight: w = raw-lower-blocks + exp(diag-blocks); out = (x @ w.T)*s + b
  with s[r] = exp(dw[r]) * rsqrt(wsn[r]), wsn = row sums of w^2
  (disjoint masks => wsn = colsums of combined^2 in the W^T layout).
  Jacobian log-det chain in linear domain:
    E0 = wn0_diag*(1-h0^2); E1 = (blockdiag(wn1)@E0)*(1-h1^2)
    E2 = blockdiag(wn2)@E1
    ldj = sum_d [ln(1 + e^gate * E2) - ln(1 + e^gate)] (flows 0,1);
          sum_d ln E2 (flow 2)
"""
import numpy as np
import ml_dtypes

DIM, HID, B = 64, 32, 1024
NCORES = 8
BC = B // NCORES
DH = DIM * HID             # 2048
NK = DH // 128             # 16
LOG_2PI = float(np.log(2.0 * np.pi))
NEG = -1e30
BF16 = ml_dtypes.bfloat16
MAIN_DT = 'bfloat16'

_CACHE = {}


# ---------------------------------------------------------------- host prep
def _host_prep(inputs):
    fl = {}
    r = np.arange(DH)
    blk = r // HID
    c64 = np.arange(DIM)
    i128 = np.arange(128)
    low128 = (i128[:, None] // HID) < (i128[None, :] // HID)
    dia128 = (i128[:, None] // HID) == (i128[None, :] // HID)
    for f in range(3):
        W0 = np.asarray(inputs[f'W{f}_0'], np.float32)
        W1 = np.asarray(inputs[f'W{f}_1'], np.float32)
        W2 = np.asarray(inputs[f'W{f}_2'], np.float32)

        W0T = W0.T                                            # (64, 2048)
        keep = c64[:, None] < blk[None, :]
        diag = c64[:, None] == blk[None, :]
        w0raw = np.where(keep, W0T, 0.0)
        w0dg = np.where(diag, W0T, NEG)
        if f > 0:
            # fold the inter-flow flip permutation into the contraction rows
            w0raw = w0raw[::-1]
            w0dg = w0dg[::-1]
        fl[f'w0s{f}'] = np.ascontiguousarray(
            np.concatenate([w0raw, w0dg], 0)).astype(BF16)    # (128, 2048)

        W1T = np.ascontiguousarray(W1.T)                      # (2048, 2048)
        fl[f'w1cc{f}'] = np.concatenate(
            [W1T[128 * k:128 * k + 128, 128 * (k + 1):]
             for k in range(NK - 1)], axis=1).astype(BF16)    # (128, 15360)
        d_raw = np.zeros((128, DH), np.float32)
        d_dg = np.full((128, DH), NEG, np.float32)
        for k in range(NK):
            t = W1T[128 * k:128 * k + 128, 128 * k:128 * k + 128]
            d_raw[:, 128 * k:128 * k + 128] = np.where(low128, t, 0.0)
            d_dg[:, 128 * k:128 * k + 128] = np.where(dia128, t, NEG)
        fl[f'w1dd{f}'] = np.concatenate([d_raw, d_dg], 1).astype(BF16)  # (128, 4096)

        W2T = np.ascontiguousarray(W2.T)                      # (2048, 64)
        keep2 = c64[None, :] > blk[:, None]
        diag2 = c64[None, :] == blk[:, None]
        w2raw = np.where(keep2, W2T, 0.0)
        w2dg = np.where(diag2, W2T, NEG)

        def gath(a):
            return np.ascontiguousarray(
                a.reshape(NK, 128, DIM).transpose(1, 0, 2).reshape(128, NK * DIM))
        fl[f'w2rd{f}'] = np.concatenate(
            [gath(w2raw), gath(w2dg)], 1).astype(BF16)        # (128, 2048)

        # packed small params: (128, 83) f32
        # cols 0:16 dw0 | 16:32 dw1 | 32:48 b0 | 48:64 b1 | 64:80 g0c
        # col 80 top: dw2 | col 81 top: b2 | col 82 top: gate (flows 0,1)
        pk = np.zeros((128, 83), np.float32)
        pk[:, 0:16] = np.asarray(inputs[f'dw{f}_0'], np.float32)[:, 0].reshape(NK, 128).T
        pk[:, 16:32] = np.asarray(inputs[f'dw{f}_1'], np.float32)[:, 0].reshape(NK, 128).T
        pk[:, 32:48] = np.asarray(inputs[f'b{f}_0'], np.float32).reshape(NK, 128).T
        pk[:, 48:64] = np.asarray(inputs[f'b{f}_1'], np.float32).reshape(NK, 128).T
        pk[:, 64:80] = W0[np.arange(DH), blk].reshape(NK, 128).T
        pk[:DIM, 80] = np.asarray(inputs[f'dw{f}_2'], np.float32).reshape(DIM)
        pk[:DIM, 81] = np.asarray(inputs[f'b{f}_2'], np.float32).reshape(DIM)
        if f < 2:
            pk[:DIM, 82] = float(np.asarray(inputs[f'gate{f}'])[0])
        fl[f'pk{f}'] = pk
    fl['flip64'] = np.eye(DIM, dtype=np.float32)[:, ::-1].astype(BF16).copy()
    fl['ones64'] = np.ones((DIM, 1), np.float32)
    fl['onescol'] = np.ones((128, 1), np.float32).astype(BF16)
    return fl


# ------------------------------------------------- walrus sync-wait splitter
def _split_sync_waits(nc, max_waits=1):
    import concourse.mybir as mybir
    for func in nc.m.functions:
        for blkb in func.blocks:
            insts = list(blkb.instructions)
            out = []
            changed = False
            for inst in insts:
                si = inst.sync_info
                if si is not None and len(si.on_wait) > max_waits:
                    waits = list(si.on_wait)
                    keep, pre = waits[-max_waits:], waits[:-max_waits]
                    chunks = [pre[i:i + max_waits] for i in range(0, len(pre), max_waits)]
                    for ci, chunk in enumerate(chunks):
                        nop = mybir.InstNoOp(name=f"{inst.name}.w{ci}", ins=[], outs=[])
                        nop.engine = inst.engine
                        nop.sync_info = mybir.SyncInfo(on_wait=chunk, on_update=[])
                        out.append(nop)
                    inst.sync_info = mybir.SyncInfo(
                        on_wait=keep, on_update=list(si.on_update))
                    changed = True
                out.append(inst)
            if changed:
                try:
                    blkb.instructions = out
                except Exception:
                    while len(blkb.instructions):
                        blkb.remove_instruction(blkb.instructions[-1])
                    for i2 in out:
                        blkb.add_instruction(i2)


# ---------------------------------------------------------------- bass build
def _build(main_dt_name='bfloat16', repeat=1):
    import concourse.bass as bass
    import concourse.mybir as mybir
    import concourse.tile as tile

    f32 = mybir.dt.float32
    bf16 = mybir.dt.bfloat16
    AO = mybir.AluOpType
    AF = mybir.ActivationFunctionType

    nc = bass.Bass()
    P = {}

    def dram(name, shape, dt=bf16):
        P[name] = nc.declare_dram_parameter(name, list(shape), dt, isOutput=False)

    WOFF = [0]
    for k in range(NK - 1):
        WOFF.append(WOFF[-1] + DH - 128 * (k + 1))            # chunk col offsets

    dram('xT', (DIM, BC))
    for f in range(3):
        dram(f'w0s{f}', (128, DH))
        dram(f'w1cc{f}', (128, WOFF[-1]))
        dram(f'w1dd{f}', (128, 2 * DH))
        dram(f'w2rd{f}', (128, 2 * NK * DIM))
        dram(f'pk{f}', (128, 83), f32)
    dram('flip64', (DIM, DIM)); dram('ones64', (DIM, 1), f32)
    dram('onescol', (128, 1))
    out = nc.declare_dram_parameter('out', [1, BC], f32, isOutput=True)

    with tile.TileContext(nc) as tc:
        with tc.tile_pool(name='const', bufs=1) as cpool, \
             tc.tile_pool(name='wchunk', bufs=2) as wpool, \
             tc.tile_pool(name='acts', bufs=2) as apool, \
             tc.tile_pool(name='hs', bufs=1) as hpool, \
             tc.tile_pool(name='sq', bufs=1) as sqpool, \
             tc.tile_pool(name='small', bufs=2) as spool, \
             tc.tile_pool(name='psz', bufs=4, space='PSUM') as pp, \
             tc.tile_pool(name='pse', bufs=2, space='PSUM') as pp1, \
             tc.tile_pool(name='psw', bufs=1, space='PSUM') as ppw:

            def mload(pool, tag, shape, dt, srcap, bufs=None, eng=None):
                t = pool.tile(list(shape), dt, name=tag, tag=tag, bufs=bufs)
                (eng or nc.sync).dma_start(out=t[:], in_=srcap)
                return t

            onescol_bf = mload(cpool, 'onescol_bf', (128, 1), bf16, P['onescol'][:])
            ones1bc = cpool.tile([DIM, BC], f32, name='ones1bc', tag='ones1bc')
            nc.vector.memset(ones1bc[:], 1.0)
            ones64 = mload(cpool, 'ones64', (DIM, 1), f32, P['ones64'][:])
            ones64b = cpool.tile([DIM, 1], bf16, name='ones64b', tag='ones64b')
            nc.vector.tensor_copy(ones64b[:], ones64[:])
            flip64 = mload(cpool, 'flip64', (DIM, DIM), bf16, P['flip64'][:])

            for _rep in range(repeat):
              x2 = cpool.tile([128, BC], bf16, name='x2', tag='x2', bufs=2)
              nc.sync.dma_start(out=x2[:DIM, :], in_=P['xT'][:])
              nc.sync.dma_start(out=x2[DIM:, :], in_=P['xT'][:])
              xgate = x2[:DIM, :]
              logp = cpool.tile([DIM, BC], bf16, name='logp', tag='logp')
              E2all = cpool.tile([DIM, 3 * BC], f32, name='E2all', tag='E2all')
              E2s = [E2all[:, f * BC:(f + 1) * BC] for f in range(3)]
              egs = [cpool.tile([DIM, 1], f32, name=f'eg_{f}', tag=f'eg_{f}')
                     for f in range(2)]

              for f in range(3):
                    # ---- loads: small tensors on Pool SWDGE queue, w1cc split
                    # into 3 segments on the sync queue so consumers pipeline
                    pkt = mload(spool, 'pk', (128, 83), f32, P[f'pk{f}'][:],
                                eng=nc.gpsimd)
                    w0s = mload(apool, 'w0s', (128, DH), bf16, P[f'w0s{f}'][:],
                                eng=nc.gpsimd)
                    w1dd = mload(apool, 'w1dd', (128, 2 * DH), bf16, P[f'w1dd{f}'][:],
                                 eng=nc.gpsimd)
                    w1draw = w1dd[:, :DH]
                    w1dg = w1dd[:, DH:]
                    w1cc = wpool.tile([128, WOFF[-1]], bf16, name='w1cc', tag='w1cc')
                    SEG = [0, 3, 7, NK - 1]                   # chunk-range segments
                    for si in range(3):
                        a, b = WOFF[SEG[si]], WOFF[SEG[si + 1]]
                        nc.sync.dma_start(out=w1cc[:, a:b],
                                          in_=P[f'w1cc{f}'][:, a:b])
                    w1chunks = [w1cc[:, WOFF[k]:WOFF[k] + DH - 128 * (k + 1)]
                                for k in range(NK - 1)]
                    w2rd = mload(apool, 'w2rd', (128, 2 * NK * DIM), bf16,
                                 P[f'w2rd{f}'][:], eng=nc.gpsimd)
                    w2raw = w2rd[:, :NK * DIM]
                    w2dg = w2rd[:, NK * DIM:]
                    dw0 = pkt[:, 0:16]
                    dw1 = pkt[:, 16:32]
                    b0 = pkt[:, 32:48]
                    b1 = pkt[:, 48:64]
                    g0c = pkt[:, 64:80]
                    dw2 = pkt[:DIM, 80:81]
                    b2 = pkt[:DIM, 81:82]
                    gc = pkt[:DIM, 82:83]

                    # ---- device exps / combines
                    edw0 = spool.tile([128, NK], f32, name='edw0', tag='edw0')
                    nc.scalar.activation(edw0[:], dw0, AF.Exp)
                    edw1 = spool.tile([128, NK], f32, name='edw1', tag='edw1')
                    nc.scalar.activation(edw1[:], dw1, AF.Exp)
                    edw2 = spool.tile([DIM, 1], f32, name='edw2', tag='edw2')
                    nc.scalar.activation(edw2[:], dw2, AF.Exp)
                    nc.scalar.activation(w0s[DIM:, :], w0s[DIM:, :], AF.Exp)
                    w1de = apool.tile([128, DH], bf16, name='w1de', tag='w1de')
                    nc.scalar.activation(w1de[:], w1dg[:], AF.Exp)
                    w1dc = apool.tile([128, DH], bf16, name='w1dc', tag='w1dc')
                    nc.vector.tensor_tensor(w1dc[:], w1draw[:], w1de[:], AO.add)
                    w2e = apool.tile([128, NK * DIM], bf16, name='w2e', tag='w2e')
                    nc.scalar.activation(w2e[:], w2dg[:], AF.Exp)
                    w2c = apool.tile([128, NK * DIM], bf16, name='w2c', tag='w2c')
                    nc.vector.tensor_tensor(w2c[:], w2raw[:], w2e[:], AO.add)

                    # ---- squares (bf16) for weight norms; sq0/sq2 chunked on
                    # DVE so the wsn matmuls (gating s0/s2 -> tanh) start early
                    sq0 = sqpool.tile([128, DH], bf16, name='sq0', tag='sq0')
                    for q in range(4):
                        (nc.vector if q % 2 == 0 else nc.gpsimd).tensor_tensor(
                            sq0[:, 512 * q:512 * q + 512],
                            w0s[:, 512 * q:512 * q + 512],
                            w0s[:, 512 * q:512 * q + 512], AO.mult)
                    sqd1 = sqpool.tile([128, DH], bf16, name='sqd1', tag='sqd1')
                    nc.vector.tensor_tensor(sqd1[:], w1dc[:], w1dc[:], AO.mult)
                    sq2 = sqpool.tile([128, NK * DIM], bf16, name='sq2', tag='sq2')
                    for q in range(2):
                        (nc.vector if q % 2 == 0 else nc.gpsimd).tensor_tensor(
                            sq2[:, 512 * q:512 * q + 512],
                            w2c[:, 512 * q:512 * q + 512],
                            w2c[:, 512 * q:512 * q + 512], AO.mult)

                    # wsn accumulators share one PSUM bank:
                    # 0:16 wsn0 | 16:32 wsn1 off-diag | 32:48 wsn1 diag | 48 wsn2
                    wsnall = ppw.tile([128, 49], f32, name='wsnall', tag='wsnall')
                    wsn0p = wsnall[:, 0:NK]
                    wsn1p = wsnall[:, NK:2 * NK]
                    wsn1d = wsnall[:, 2 * NK:3 * NK]
                    wsn2p = wsnall[:DIM, 48:49]
                    for m in range(NK):
                        nc.tensor.matmul(wsn0p[:, m:m + 1], sq0[:, 128 * m:128 * m + 128],
                                         onescol_bf[:], start=True, stop=True)
                    s0 = _scol(nc, spool, AO, AF, wsn0p, edw0, (128, NK), 's0')

                    # wsn1: off-diag chunk squares + col sums, accumulated in a
                    # single PSUM tile via per-column start/stop groups
                    sq1 = []
                    for k in range(NK - 1):
                        s = sqpool.tile([128, DH - 128 * (k + 1)], bf16,
                                        name=f'sq1t{k}', tag=f'sq1t{k}')
                        eng = nc.vector if k < 8 else nc.gpsimd
                        eng.tensor_tensor(s[:], w1chunks[k][:],
                                          w1chunks[k][:], AO.mult)
                        sq1.append(s)
                    # column-major off-diag groups (close without sqd1); the
                    # diag contributions are independent one-shots emitted after
                    for m in range(1, NK):
                        for k in range(m):
                            off = 128 * (m - k - 1)
                            nc.tensor.matmul(wsn1p[:, m:m + 1],
                                             sq1[k][:, off:off + 128],
                                             onescol_bf[:], start=(k == 0),
                                             stop=(k == m - 1))
                    for m in range(NK):
                        nc.tensor.matmul(wsn1d[:, m:m + 1], sqd1[:, 128 * m:128 * m + 128],
                                         onescol_bf[:], start=True, stop=True)
                    s1 = _scol(nc, spool, AO, AF, wsn1p, edw1, (128, NK), 's1',
                               extra=wsn1d)

                    # wsn2: 16 N=1 K=128 M=64 accumulating matmuls
                    for k in range(NK):
                        nc.tensor.matmul(wsn2p[:], sq2[:, k * DIM:(k + 1) * DIM],
                                         onescol_bf[:], start=(k == 0),
                                         stop=(k == NK - 1))
                    s2 = _scol(nc, spool, AO, AF, wsn2p, edw2, (DIM, 1), 's2')

                    # gate exps early (keep Act in the exp/ln table phase)
                    if f < 2:
                        nc.scalar.activation(egs[f][:], gc[:], AF.Exp)

                    # c0 = s0 * exp(g0raw); and its negation
                    eg0 = spool.tile([128, NK], f32, name='eg0', tag='eg0')
                    nc.scalar.activation(eg0[:], g0c[:], AF.Exp)
                    c0 = spool.tile([128, NK], f32, name='c0', tag='c0')
                    nc.vector.tensor_tensor(c0[:], eg0[:], s0[:], AO.mult)
                    c0n = spool.tile([128, NK], f32, name='c0n', tag='c0n')
                    nc.vector.tensor_scalar(out=c0n[:], in0=c0[:], scalar1=-1.0,
                                            scalar2=None, op0=AO.mult)
                    s1n = spool.tile([128, NK], f32, name='s1n', tag='s1n')
                    nc.vector.tensor_scalar(out=s1n[:], in0=s1[:], scalar1=-1.0,
                                            scalar2=None, op0=AO.mult)

                    # ---- layer 0: z0 = w0s.T @ [x; x]  (K=128 covers raw+exp)
                    h0a = hpool.tile([128, DH], bf16, name='h0a', tag='h0a')
                    E0a = hpool.tile([128, DH], bf16, name='E0a', tag='E0a')
                    h0 = [h0a[:, 128 * m:128 * m + 128] for m in range(NK)]
                    E0 = [E0a[:, 128 * m:128 * m + 128] for m in range(NK)]
                    for g in range(4):
                        ps = pp.tile([128, 512], f32, name='zmain', tag='zmain')
                        for j in range(4):
                            m = 4 * g + j
                            nc.tensor.matmul(ps[:, 128 * j:128 * j + 128],
                                             w0s[:, 128 * m:128 * m + 128], x2[:],
                                             start=True, stop=True)
                        for j in range(4):
                            m = 4 * g + j
                            sl = ps[:, 128 * j:128 * j + 128]
                            nc.scalar.activation(h0[m], sl, AF.Tanh,
                                                 bias=b0[:, m:m + 1], scale=s0[:, m:m + 1])
                        hsq = spool.tile([128, 512], bf16, name='hsq', tag='hsq')
                        nc.vector.tensor_tensor(hsq[:], h0a[:, 512 * g:512 * g + 512],
                                                h0a[:, 512 * g:512 * g + 512], AO.mult)
                        for j in range(4):
                            m = 4 * g + j
                            nc.gpsimd.tensor_scalar(out=E0[m], in0=hsq[:, 128 * j:128 * j + 128],
                                                    scalar1=c0n[:, m:m + 1],
                                                    scalar2=c0[:, m:m + 1],
                                                    op0=AO.mult, op1=AO.add)

                    # ---- layer 1 (+ E-path combine)
                    h1a = hpool.tile([128, DH], bf16, name='h1a', tag='h1a')
                    E1a = hpool.tile([128, DH], bf16, name='E1a', tag='E1a')
                    h1 = [h1a[:, 128 * m:128 * m + 128] for m in range(NK)]
                    E1 = [E1a[:, 128 * m:128 * m + 128] for m in range(NK)]
                    for g in range(4):
                        ps = pp.tile([128, 512], f32, name='zmain', tag='zmain')
                        for j in range(4):
                            m = 4 * g + j
                            sl = ps[:, 128 * j:128 * j + 128]
                            for k in range(m):
                                off = 128 * (m - k - 1)
                                nc.tensor.matmul(
                                    sl, w1chunks[k][:, off:off + 128],
                                    h0[k], start=(k == 0), stop=False)
                            nc.tensor.matmul(sl, w1dc[:, 128 * m:128 * m + 128],
                                             h0[m], start=(m == 0), stop=True)
                        psE = pp1.tile([128, 512], f32, name='epath', tag='epath')
                        for j in range(4):
                            m = 4 * g + j
                            nc.tensor.matmul(psE[:, 128 * j:128 * j + 128],
                                             w1de[:, 128 * m:128 * m + 128], E0[m],
                                             start=True, stop=True)
                        for j in range(4):
                            m = 4 * g + j
                            sl = ps[:, 128 * j:128 * j + 128]
                            nc.scalar.activation(h1[m], sl, AF.Tanh,
                                                 bias=b1[:, m:m + 1], scale=s1[:, m:m + 1])
                        hsq = spool.tile([128, 512], bf16, name='hsq', tag='hsq')
                        nc.vector.tensor_tensor(hsq[:], h1a[:, 512 * g:512 * g + 512],
                                                h1a[:, 512 * g:512 * g + 512], AO.mult)
                        etsw = spool.tile([128, 512], bf16, name='etsw', tag='etsw')
                        for j in range(4):
                            m = 4 * g + j
                            nc.gpsimd.tensor_scalar(out=etsw[:, 128 * j:128 * j + 128],
                                                    in0=hsq[:, 128 * j:128 * j + 128],
                                                    scalar1=s1n[:, m:m + 1],
                                                    scalar2=s1[:, m:m + 1],
                                                    op0=AO.mult, op1=AO.add)
                        nc.vector.tensor_tensor(E1a[:, 512 * g:512 * g + 512],
                                                psE[:, :], etsw[:], AO.mult)

                    # ---- layer 2
                    psz2 = ppw.tile([DIM, BC], f32, name='zsmall', tag='zsmall')
                    for k in range(NK):
                        nc.tensor.matmul(psz2[:], w2c[:, k * DIM:(k + 1) * DIM], h1[k],
                                         start=(k == 0), stop=(k == NK - 1))
                    z2 = spool.tile([DIM, BC], f32, name='z2s', tag='z2s')
                    nc.scalar.activation(z2[:], psz2[:], AF.Identity,
                                         bias=b2[:, 0:1], scale=s2[:, 0:1])
                    psE2 = pp1.tile([DIM, BC], f32, name='epath2', tag='epath')
                    for k in range(NK):
                        nc.tensor.matmul(psE2[:], w2e[:, k * DIM:(k + 1) * DIM],
                                         E1[k], start=(k == 0), stop=(k == NK - 1))
                    nc.vector.tensor_scalar(out=E2s[f][:], in0=psE2[:],
                                            scalar1=s2[:, 0:1], scalar2=None,
                                            op0=AO.mult)
                    if f < 2:
                        nc.vector.tensor_scalar(out=E2s[f][:], in0=E2s[f][:],
                                                scalar1=egs[f][:], scalar2=1.0,
                                                op0=AO.mult, op1=AO.add)

                    # ---- gate mix / flip or final logp term
                    if f < 2:
                        th = spool.tile([DIM, 1], f32, name='th', tag='th')
                        nc.scalar.activation(th[:], gc[:], AF.Tanh, scale=0.5)
                        sg = spool.tile([DIM, 1], f32, name='sg', tag='sg')
                        nc.vector.tensor_scalar(out=sg[:], in0=th[:], scalar1=0.5,
                                                scalar2=0.5, op0=AO.mult, op1=AO.add)
                        # xmix = sg*(z2 - x) + x; the next flow consumes xmix
                        # directly (flip folded into its w0s row order); the
                        # flipped copy is only needed for the next gate mix,
                        # built off the critical path
                        td = spool.tile([DIM, BC], f32, name='td', tag='td')
                        nc.vector.tensor_tensor(td[:], z2[:], xgate, AO.subtract)
                        nc.vector.tensor_scalar(out=td[:], in0=td[:], scalar1=sg[:],
                                                scalar2=None, op0=AO.mult)
                        x2 = cpool.tile([128, BC], bf16, name='x2', tag='x2',
                                        bufs=2)
                        nc.vector.tensor_tensor(x2[:DIM, :], td[:], xgate, AO.add)
                        nc.scalar.activation(x2[DIM:, :], x2[:DIM, :], AF.Copy)
                        psf = ppw.tile([DIM, BC], f32, name='zsmall', tag='zsmall')
                        nc.tensor.matmul(psf[:], flip64[:], x2[:DIM, :],
                                         start=True, stop=True)
                        xflip = spool.tile([DIM, BC], bf16, name='xflip', tag='xflip')
                        nc.scalar.activation(xflip[:], psf[:], AF.Copy)
                        xgate = xflip[:]
                    else:
                        sqx = spool.tile([DIM, BC], f32, name='sqx', tag='sqx')
                        nc.gpsimd.tensor_tensor(sqx[:], z2[:], z2[:], AO.mult)
                        nc.vector.tensor_scalar(out=logp[:], in0=sqx[:], scalar1=-0.5,
                                                scalar2=-0.5 * LOG_2PI, op0=AO.mult,
                                                op1=AO.add)

              # ---- ldj tail: l1p ln first (absorbs the act-table load in
              # an idle slot), then one wide ln; PE accumulates all feature
              # sums directly into the (1, BC) output PSUM
              l1p = spool.tile([DIM, 2], f32, name='l1p', tag='l1p')
              for f in range(2):
                    nc.vector.tensor_scalar(out=l1p[:, f:f + 1], in0=egs[f][:],
                                            scalar1=1.0, scalar2=None, op0=AO.add)
              nc.scalar.activation(l1p[:], l1p[:], AF.Ln)
              negp = spool.tile([DIM, 1], f32, name='negp', tag='negp')
              nc.vector.tensor_tensor(negp[:], l1p[:, 0:1], l1p[:, 1:2], AO.add)
              nc.vector.tensor_scalar(out=negp[:], in0=negp[:], scalar1=-1.0,
                                      scalar2=None, op0=AO.mult)
              lfall = spool.tile([DIM, 3 * BC], bf16, name='lfall', tag='lfall')
              nc.scalar.activation(lfall[:], E2all[:], AF.Ln)
              psum_out = ppw.tile([1, BC], f32, name='zsmall', tag='zsmall')
              nc.tensor.matmul(psum_out[:], ones64b[:], logp[:], start=True, stop=False)
              for f in range(3):
                    nc.tensor.matmul(psum_out[:], ones64b[:],
                                     lfall[:, f * BC:(f + 1) * BC],
                                     start=False, stop=False)
              nc.tensor.matmul(psum_out[:], negp[:], ones1bc[:],
                               start=False, stop=True)
              outs = spool.tile([1, BC], f32, name='outs', tag='outs')
              nc.vector.tensor_copy(outs[:], psum_out[:])
              nc.sync.dma_start(out=out[:], in_=outs[:])

    _split_sync_waits(nc)
    return nc


def _scol(nc, spool, AO, AF, wsnp, edw, shape, tag, iters=None, extra=None):
    """s = exp(dw) * rsqrt(wsn) via DVE bit-trick + Newton (no Ln/Exp)."""
    if iters is None:
        iters = RSQRT_ITERS
    import concourse.mybir as mybir
    f32 = mybir.dt.float32
    u32 = mybir.dt.uint32
    w = spool.tile(list(shape), f32, name=tag + 'w')
    if extra is None:
        nc.vector.tensor_copy(w[:], wsnp[:])
    else:
        # only one PSUM operand per DVE op: copy extra (all cols valid),
        # then add the main region (col 0 has no off-diag writes)
        nc.vector.tensor_copy(w[:], extra[:])
        nc.vector.tensor_tensor(w[:, 1:], w[:, 1:], wsnp[:, 1:], AO.add)
    y = spool.tile(list(shape), u32, name=tag + 'y')
    nc.vector.tensor_scalar(out=y[:], in0=w[:].bitcast(u32), scalar1=1,
                            scalar2=None, op0=AO.logical_shift_right)
    nc.vector.tensor_scalar(out=y[:], in0=y[:], scalar1=-1, scalar2=0x5f3759df,
                            op0=AO.mult, op1=AO.add)
    yf = y[:].bitcast(f32)
    a = spool.tile(list(shape), f32, name=tag + 'a')
    for _ in range(iters):
        nc.vector.tensor_tensor(a[:], yf, yf, AO.mult)
        nc.vector.tensor_tensor(a[:], a[:], w[:], AO.mult)
        nc.vector.tensor_scalar(out=a[:], in0=a[:], scalar1=-0.5, scalar2=1.5,
                                op0=AO.mult, op1=AO.add)
        nc.vector.tensor_tensor(yf, yf, a[:], AO.mult)
    s = spool.tile(list(shape), f32, name=tag + 's')
    nc.vector.tensor_tensor(s[:], yf, edw[:], AO.mult)
    return s


RSQRT_ITERS = 2


# ------------------------------------------------------------------ runner
def _make_runner(nc, n_cores):
    import jax
    from jax.sharding import Mesh, PartitionSpec
    from jax.experimental.shard_map import shard_map
    import concourse.mybir as mybir
    from concourse.bass2jax import (_bass_exec_p, partition_id_tensor,
                                    install_neuronx_cc_hook)
    install_neuronx_cc_hook()
    partition_name = nc.partition_id_tensor.name if nc.partition_id_tensor else None
    in_names, out_names, out_avals = [], [], []
    for alloc in nc.m.functions[0].allocations:
        if not isinstance(alloc, mybir.MemoryLocationSet):
            continue
        name = alloc.memorylocations[0].name
        if alloc.kind == "ExternalInput":
            if name != partition_name:
                in_names.append(name)
        elif alloc.kind == "ExternalOutput":
            out_names.append(name)
            out_avals.append(jax.core.ShapedArray(
                tuple(alloc.tensor_shape), mybir.dt.np(alloc.dtype)))
    n_params = len(in_names)
    all_names = in_names + out_names + ([partition_name] if partition_name else [])

    def _body(*args):
        operands = list(args)
        if partition_name is not None:
            operands.append(partition_id_tensor())
        outs = _bass_exec_p.bind(
            *operands, out_avals=tuple(out_avals), in_names=tuple(all_names),
            out_names=tuple(out_names), lowering_input_output_aliases=(),
            sim_require_finite=False, sim_require_nnan=False, nc=nc)
        return tuple(outs)

    devices = jax.devices()[:n_cores]
    mesh = Mesh(np.asarray(devices), ("core",))
    n_outs = len(out_names)
    in_specs = (PartitionSpec("core"),) * (n_params + n_outs)
    out_specs = (PartitionSpec("core"),) * n_outs
    fn = jax.jit(shard_map(_body, mesh=mesh, in_specs=in_specs,
                           out_specs=out_specs, check_rep=False),
                 keep_unused=True)
    return fn, in_names, out_names, out_avals


def _get_runner():
    key = ('runner', MAIN_DT)
    if key not in _CACHE:
        import sys, os
        d = os.path.dirname(os.path.abspath(__file__))
        if d not in sys.path:
            sys.path.insert(0, d)
        nc = _build(MAIN_DT)
        _CACHE[key] = _make_runner(nc, NCORES)
    return _CACHE[key]


def kernel(**inputs):
    fl = _host_prep(inputs)
    x = np.asarray(inputs['x'])
    fn, in_names, out_names, out_avals = _get_runner()
    in_maps = []
    for c in range(NCORES):
        m = dict(fl)
        m['xT'] = np.ascontiguousarray(x[c * BC:(c + 1) * BC, :].T).astype(BF16)
        in_maps.append(m)
    concat_in = [np.concatenate([np.asarray(m[name]) for m in in_maps], axis=0)
                 for name in in_names]
    concat_zeros = [np.zeros((NCORES * a.shape[0], *a.shape[1:]), a.dtype)
                    for a in out_avals]
    outs = fn(*concat_in, *concat_zeros)
    o = np.asarray(outs[0]).reshape(NCORES, BC)
    return o.reshape(B).astype(np.float32)


# revision 10
# speedup vs baseline: 6.7024x; 3.5612x over previous
"""BNAF forward + log-det on 8 TRN2 NeuronCores (self-contained), v2.

Sharding: data-parallel over batch (128 rows/core), params replicated.
Host does layout-only prep (transpose / slice / gather / structural masking /
constant fills / dtype staging to bf16); all arithmetic on input values
happens on device.

Math (validated vs reference, rel err ~1e-3 vs 2e-2 gate):
  masked weight: w = raw-lower-blocks + exp(diag-blocks); out = (x @ w.T)*s + b
  with s[r] = exp(dw[r]) * rsqrt(wsn[r]), wsn = row sums of w^2
  (disjoint masks => wsn = colsums of combined^2 in the W^T layout).
  rsqrt runs on the DVE via bitcast + Newton so the whole flow body stays on
  one activation table (exp/tanh/square co-resident; no Ln until the tail).
  Jacobian log-det chain in linear domain:
    E0 = wn0_diag*(1-h0^2); E1 = (blockdiag(wn1)@E0)*(1-h1^2)
    E2 = blockdiag(wn2)@E1
    ldj = sum_d [ln(1 + e^gate * E2) - ln(1 + e^gate)] (flows 0,1);
          sum_d ln E2 (flow 2)
  The inter-flow flip permutation is folded into the host-side row order of
  the next flow's layer-0 weights; the flipped activation tensor itself is
  only materialized off the critical path for the gate mix.

Layout/perf notes:
  - all matmul operands bf16 (fp32 PSUM accumulation); weights ship as bf16
  - layer-0 raw and exp(diag) halves are stacked into one K=128 stationary
  - w1 off-diag chunks are pure upper-triangle slices (no masking needed),
    packed into a single DRAM tensor, loaded in 3 segments for pipelining
  - weight-norm column sums run as N=1 matmuls over bf16 squared tiles,
    PSUM-accumulated with bank-safe column-major groups
  - elementwise work is split across DVE/Pool; GPSIMD never touches PSUM
    and every instruction carries at most one sync wait (walrus limits)
"""
import numpy as np
import ml_dtypes

DIM, HID, B = 64, 32, 1024
NCORES = 8
BC = B // NCORES
DH = DIM * HID             # 2048
NK = DH // 128             # 16
LOG_2PI = float(np.log(2.0 * np.pi))
NEG = -1e30
BF16 = ml_dtypes.bfloat16
MAIN_DT = 'bfloat16'

_CACHE = {}


# ---------------------------------------------------------------- host prep
def _host_prep(inputs):
    fl = {}
    r = np.arange(DH)
    blk = r // HID
    c64 = np.arange(DIM)
    i128 = np.arange(128)
    low128 = (i128[:, None] // HID) < (i128[None, :] // HID)
    dia128 = (i128[:, None] // HID) == (i128[None, :] // HID)
    for f in range(3):
        W0 = np.asarray(inputs[f'W{f}_0'], np.float32)
        W1 = np.asarray(inputs[f'W{f}_1'], np.float32)
        W2 = np.asarray(inputs[f'W{f}_2'], np.float32)

        W0T = W0.T                                            # (64, 2048)
        keep = c64[:, None] < blk[None, :]
        diag = c64[:, None] == blk[None, :]
        w0raw = np.where(keep, W0T, 0.0)
        w0dg = np.where(diag, W0T, NEG)
        if f > 0:
            # fold the inter-flow flip permutation into the contraction rows
            w0raw = w0raw[::-1]
            w0dg = w0dg[::-1]
        fl[f'w0s{f}'] = np.ascontiguousarray(
            np.concatenate([w0raw, w0dg], 0)).astype(BF16)    # (128, 2048)

        W1T = np.ascontiguousarray(W1.T)                      # (2048, 2048)
        fl[f'w1cc{f}'] = np.concatenate(
            [W1T[128 * k:128 * k + 128, 128 * (k + 1):]
             for k in range(NK - 1)], axis=1).astype(BF16)    # (128, 15360)
        d_raw = np.zeros((128, DH), np.float32)
        d_dg = np.full((128, DH), NEG, np.float32)
        for k in range(NK):
            t = W1T[128 * k:128 * k + 128, 128 * k:128 * k + 128]
            d_raw[:, 128 * k:128 * k + 128] = np.where(low128, t, 0.0)
            d_dg[:, 128 * k:128 * k + 128] = np.where(dia128, t, NEG)
        fl[f'w1dd{f}'] = np.concatenate([d_raw, d_dg], 1).astype(BF16)  # (128, 4096)

        W2T = np.ascontiguousarray(W2.T)                      # (2048, 64)
        keep2 = c64[None, :] > blk[:, None]
        diag2 = c64[None, :] == blk[:, None]
        w2raw = np.where(keep2, W2T, 0.0)
        w2dg = np.where(diag2, W2T, NEG)

        def gath(a):
            return np.ascontiguousarray(
                a.reshape(NK, 128, DIM).transpose(1, 0, 2).reshape(128, NK * DIM))
        fl[f'w2rd{f}'] = np.concatenate(
            [gath(w2raw), gath(w2dg)], 1).astype(BF16)        # (128, 2048)

        # packed small params: (128, 83) f32
        # cols 0:16 dw0 | 16:32 dw1 | 32:48 b0 | 48:64 b1 | 64:80 g0c
        # col 80 top: dw2 | col 81 top: b2 | col 82 top: gate (flows 0,1)
        pk = np.zeros((128, 83), np.float32)
        pk[:, 0:16] = np.asarray(inputs[f'dw{f}_0'], np.float32)[:, 0].reshape(NK, 128).T
        pk[:, 16:32] = np.asarray(inputs[f'dw{f}_1'], np.float32)[:, 0].reshape(NK, 128).T
        pk[:, 32:48] = np.asarray(inputs[f'b{f}_0'], np.float32).reshape(NK, 128).T
        pk[:, 48:64] = np.asarray(inputs[f'b{f}_1'], np.float32).reshape(NK, 128).T
        pk[:, 64:80] = W0[np.arange(DH), blk].reshape(NK, 128).T
        pk[:DIM, 80] = np.asarray(inputs[f'dw{f}_2'], np.float32).reshape(DIM)
        pk[:DIM, 81] = np.asarray(inputs[f'b{f}_2'], np.float32).reshape(DIM)
        if f < 2:
            pk[:DIM, 82] = float(np.asarray(inputs[f'gate{f}'])[0])
        fl[f'pk{f}'] = pk
    fl['flip64'] = np.eye(DIM, dtype=np.float32)[:, ::-1].astype(BF16).copy()
    fl['ones64'] = np.ones((DIM, 1), np.float32)
    fl['onescol'] = np.ones((128, 1), np.float32).astype(BF16)
    return fl


# ------------------------------------------------- walrus sync-wait splitter
def _split_sync_waits(nc, max_waits=1):
    import concourse.mybir as mybir
    for func in nc.m.functions:
        for blkb in func.blocks:
            insts = list(blkb.instructions)
            out = []
            changed = False
            for inst in insts:
                si = inst.sync_info
                if si is not None and len(si.on_wait) > max_waits:
                    waits = list(si.on_wait)
                    keep, pre = waits[-max_waits:], waits[:-max_waits]
                    chunks = [pre[i:i + max_waits] for i in range(0, len(pre), max_waits)]
                    for ci, chunk in enumerate(chunks):
                        nop = mybir.InstNoOp(name=f"{inst.name}.w{ci}", ins=[], outs=[])
                        nop.engine = inst.engine
                        nop.sync_info = mybir.SyncInfo(on_wait=chunk, on_update=[])
                        out.append(nop)
                    inst.sync_info = mybir.SyncInfo(
                        on_wait=keep, on_update=list(si.on_update))
                    changed = True
                out.append(inst)
            if changed:
                try:
                    blkb.instructions = out
                except Exception:
                    while len(blkb.instructions):
                        blkb.remove_instruction(blkb.instructions[-1])
                    for i2 in out:
                        blkb.add_instruction(i2)


# ---------------------------------------------------------------- bass build
def _build(main_dt_name='bfloat16', repeat=1):
    import concourse.bass as bass
    import concourse.mybir as mybir
    import concourse.tile as tile

    f32 = mybir.dt.float32
    bf16 = mybir.dt.bfloat16
    AO = mybir.AluOpType
    AF = mybir.ActivationFunctionType

    nc = bass.Bass()
    P = {}

    def dram(name, shape, dt=bf16):
        P[name] = nc.declare_dram_parameter(name, list(shape), dt, isOutput=False)

    WOFF = [0]
    for k in range(NK - 1):
        WOFF.append(WOFF[-1] + DH - 128 * (k + 1))            # chunk col offsets

    dram('xT', (DIM, BC))
    for f in range(3):
        dram(f'w0s{f}', (128, DH))
        dram(f'w1cc{f}', (128, WOFF[-1]))
        dram(f'w1dd{f}', (128, 2 * DH))
        dram(f'w2rd{f}', (128, 2 * NK * DIM))
        dram(f'pk{f}', (128, 83), f32)
    dram('flip64', (DIM, DIM)); dram('ones64', (DIM, 1), f32)
    dram('onescol', (128, 1))
    out = nc.declare_dram_parameter('out', [1, BC], f32, isOutput=True)

    with tile.TileContext(nc) as tc:
        with tc.tile_pool(name='const', bufs=1) as cpool, \
             tc.tile_pool(name='wchunk', bufs=2) as wpool, \
             tc.tile_pool(name='acts', bufs=2) as apool, \
             tc.tile_pool(name='hs', bufs=1) as hpool, \
             tc.tile_pool(name='sq', bufs=1) as sqpool, \
             tc.tile_pool(name='small', bufs=2) as spool, \
             tc.tile_pool(name='psz', bufs=4, space='PSUM') as pp, \
             tc.tile_pool(name='pse', bufs=2, space='PSUM') as pp1, \
             tc.tile_pool(name='psw', bufs=1, space='PSUM') as ppw:

            def mload(pool, tag, shape, dt, srcap, bufs=None, eng=None):
                t = pool.tile(list(shape), dt, name=tag, tag=tag, bufs=bufs)
                (eng or nc.sync).dma_start(out=t[:], in_=srcap)
                return t

            onescol_bf = mload(cpool, 'onescol_bf', (128, 1), bf16, P['onescol'][:])
            ones1bc = cpool.tile([DIM, BC], f32, name='ones1bc', tag='ones1bc')
            nc.vector.memset(ones1bc[:], 1.0)
            ones64 = mload(cpool, 'ones64', (DIM, 1), f32, P['ones64'][:])
            ones64b = cpool.tile([DIM, 1], bf16, name='ones64b', tag='ones64b')
            nc.vector.tensor_copy(ones64b[:], ones64[:])
            flip64 = mload(cpool, 'flip64', (DIM, DIM), bf16, P['flip64'][:])

            for _rep in range(repeat):
              x2 = cpool.tile([128, BC], bf16, name='x2', tag='x2', bufs=2)
              nc.sync.dma_start(out=x2[:DIM, :], in_=P['xT'][:])
              nc.sync.dma_start(out=x2[DIM:, :], in_=P['xT'][:])
              xgate = x2[:DIM, :]
              logp = cpool.tile([DIM, BC], bf16, name='logp', tag='logp')
              E2all = cpool.tile([DIM, 3 * BC], f32, name='E2all', tag='E2all')
              E2s = [E2all[:, f * BC:(f + 1) * BC] for f in range(3)]
              egs = [cpool.tile([DIM, 1], f32, name=f'eg_{f}', tag=f'eg_{f}')
                     for f in range(2)]

              for f in range(3):
                    # ---- loads: small tensors on Pool SWDGE queue, w1cc split
                    # into 3 segments on the sync queue so consumers pipeline
                    pkt = mload(spool, 'pk', (128, 83), f32, P[f'pk{f}'][:],
                                eng=nc.gpsimd)
                    w0s = mload(apool, 'w0s', (128, DH), bf16, P[f'w0s{f}'][:],
                                eng=nc.gpsimd)
                    w1dd = mload(apool, 'w1dd', (128, 2 * DH), bf16, P[f'w1dd{f}'][:],
                                 eng=nc.gpsimd)
                    w1draw = w1dd[:, :DH]
                    w1dg = w1dd[:, DH:]
                    w1cc = wpool.tile([128, WOFF[-1]], bf16, name='w1cc', tag='w1cc')
                    SEG = [0, 2, 5, 9, NK - 1]                # chunk-range segments
                    for si in range(4):
                        a, b = WOFF[SEG[si]], WOFF[SEG[si + 1]]
                        nc.sync.dma_start(out=w1cc[:, a:b],
                                          in_=P[f'w1cc{f}'][:, a:b])
                    w1chunks = [w1cc[:, WOFF[k]:WOFF[k] + DH - 128 * (k + 1)]
                                for k in range(NK - 1)]
                    w2rd = mload(apool, 'w2rd', (128, 2 * NK * DIM), bf16,
                                 P[f'w2rd{f}'][:], eng=nc.gpsimd)
                    w2raw = w2rd[:, :NK * DIM]
                    w2dg = w2rd[:, NK * DIM:]
                    dw0 = pkt[:, 0:16]
                    dw1 = pkt[:, 16:32]
                    b0 = pkt[:, 32:48]
                    b1 = pkt[:, 48:64]
                    g0c = pkt[:, 64:80]
                    dw2 = pkt[:DIM, 80:81]
                    b2 = pkt[:DIM, 81:82]
                    gc = pkt[:DIM, 82:83]

                    # ---- device exps / combines
                    edw0 = spool.tile([128, NK], f32, name='edw0', tag='edw0')
                    nc.scalar.activation(edw0[:], dw0, AF.Exp)
                    edw1 = spool.tile([128, NK], f32, name='edw1', tag='edw1')
                    nc.scalar.activation(edw1[:], dw1, AF.Exp)
                    edw2 = spool.tile([DIM, 1], f32, name='edw2', tag='edw2')
                    nc.scalar.activation(edw2[:], dw2, AF.Exp)
                    nc.scalar.activation(w0s[DIM:, :], w0s[DIM:, :], AF.Exp)
                    w1de = apool.tile([128, DH], bf16, name='w1de', tag='w1de')
                    nc.scalar.activation(w1de[:], w1dg[:], AF.Exp)
                    w1dc = apool.tile([128, DH], bf16, name='w1dc', tag='w1dc')
                    nc.vector.tensor_tensor(w1dc[:], w1draw[:], w1de[:], AO.add)
                    w2e = apool.tile([128, NK * DIM], bf16, name='w2e', tag='w2e')
                    nc.scalar.activation(w2e[:], w2dg[:], AF.Exp)
                    w2c = apool.tile([128, NK * DIM], bf16, name='w2c', tag='w2c')
                    nc.vector.tensor_tensor(w2c[:], w2raw[:], w2e[:], AO.add)

                    # ---- squares (bf16) for weight norms; sq0/sq2 chunked on
                    # DVE so the wsn matmuls (gating s0/s2 -> tanh) start early
                    sq0 = sqpool.tile([128, DH], bf16, name='sq0', tag='sq0')
                    for q in range(4):
                        (nc.vector if q % 2 == 0 else nc.gpsimd).tensor_tensor(
                            sq0[:, 512 * q:512 * q + 512],
                            w0s[:, 512 * q:512 * q + 512],
                            w0s[:, 512 * q:512 * q + 512], AO.mult)
                    sqd1 = sqpool.tile([128, DH], bf16, name='sqd1', tag='sqd1')
                    nc.vector.tensor_tensor(sqd1[:], w1dc[:], w1dc[:], AO.mult)
                    sq2 = sqpool.tile([128, NK * DIM], bf16, name='sq2', tag='sq2')
                    for q in range(2):
                        (nc.vector if q % 2 == 0 else nc.gpsimd).tensor_tensor(
                            sq2[:, 512 * q:512 * q + 512],
                            w2c[:, 512 * q:512 * q + 512],
                            w2c[:, 512 * q:512 * q + 512], AO.mult)

                    # wsn accumulators share one PSUM bank:
                    # 0:16 wsn0 | 16:32 wsn1 off-diag | 32:48 wsn1 diag | 48 wsn2
                    wsnall = ppw.tile([128, 49], f32, name='wsnall', tag='wsnall')
                    wsn0p = wsnall[:, 0:NK]
                    wsn1p = wsnall[:, NK:2 * NK]
                    wsn1d = wsnall[:, 2 * NK:3 * NK]
                    wsn2p = wsnall[:DIM, 48:49]
                    for m in range(NK):
                        nc.tensor.matmul(wsn0p[:, m:m + 1], sq0[:, 128 * m:128 * m + 128],
                                         onescol_bf[:], start=True, stop=True)
                    s0 = _scol(nc, spool, AO, AF, wsn0p, edw0, (128, NK), 's0')

                    # wsn1: off-diag chunk squares + col sums, accumulated in a
                    # single PSUM tile via per-column start/stop groups
                    sq1 = []
                    for k in range(NK - 1):
                        s = sqpool.tile([128, DH - 128 * (k + 1)], bf16,
                                        name=f'sq1t{k}', tag=f'sq1t{k}')
                        eng = nc.vector if k < 8 else nc.gpsimd
                        eng.tensor_tensor(s[:], w1chunks[k][:],
                                          w1chunks[k][:], AO.mult)
                        sq1.append(s)
                    # column-major off-diag groups (close without sqd1); the
                    # diag contributions are independent one-shots emitted after
                    for m in range(1, NK):
                        for k in range(m):
                            off = 128 * (m - k - 1)
                            nc.tensor.matmul(wsn1p[:, m:m + 1],
                                             sq1[k][:, off:off + 128],
                                             onescol_bf[:], start=(k == 0),
                                             stop=(k == m - 1))
                    for m in range(NK):
                        nc.tensor.matmul(wsn1d[:, m:m + 1], sqd1[:, 128 * m:128 * m + 128],
                                         onescol_bf[:], start=True, stop=True)
                    s1 = _scol(nc, spool, AO, AF, wsn1p, edw1, (128, NK), 's1',
                               extra=wsn1d)

                    # wsn2: 16 N=1 K=128 M=64 accumulating matmuls
                    for k in range(NK):
                        nc.tensor.matmul(wsn2p[:], sq2[:, k * DIM:(k + 1) * DIM],
                                         onescol_bf[:], start=(k == 0),
                                         stop=(k == NK - 1))
                    s2 = _scol(nc, spool, AO, AF, wsn2p, edw2, (DIM, 1), 's2')

                    # gate exps early (keep Act in the exp/ln table phase)
                    if f < 2:
                        nc.scalar.activation(egs[f][:], gc[:], AF.Exp)

                    # c0 = s0 * exp(g0raw); and its negation
                    eg0 = spool.tile([128, NK], f32, name='eg0', tag='eg0')
                    nc.scalar.activation(eg0[:], g0c[:], AF.Exp)
                    c0 = spool.tile([128, NK], f32, name='c0', tag='c0')
                    nc.vector.tensor_tensor(c0[:], eg0[:], s0[:], AO.mult)
                    c0n = spool.tile([128, NK], f32, name='c0n', tag='c0n')
                    nc.vector.tensor_scalar(out=c0n[:], in0=c0[:], scalar1=-1.0,
                                            scalar2=None, op0=AO.mult)
                    s1n = spool.tile([128, NK], f32, name='s1n', tag='s1n')
                    nc.vector.tensor_scalar(out=s1n[:], in0=s1[:], scalar1=-1.0,
                                            scalar2=None, op0=AO.mult)

                    # ---- layer 0: z0 = w0s.T @ [x; x]  (K=128 covers raw+exp)
                    h0a = hpool.tile([128, DH], bf16, name='h0a', tag='h0a')
                    E0a = hpool.tile([128, DH], bf16, name='E0a', tag='E0a')
                    h0 = [h0a[:, 128 * m:128 * m + 128] for m in range(NK)]
                    E0 = [E0a[:, 128 * m:128 * m + 128] for m in range(NK)]
                    for g in range(4):
                        ps = pp.tile([128, 512], f32, name='zmain', tag='zmain')
                        for j in range(4):
                            m = 4 * g + j
                            nc.tensor.matmul(ps[:, 128 * j:128 * j + 128],
                                             w0s[:, 128 * m:128 * m + 128], x2[:],
                                             start=True, stop=True)
                        for j in range(4):
                            m = 4 * g + j
                            sl = ps[:, 128 * j:128 * j + 128]
                            nc.scalar.activation(h0[m], sl, AF.Tanh,
                                                 bias=b0[:, m:m + 1], scale=s0[:, m:m + 1])
                        hsq = spool.tile([128, 512], bf16, name='hsq', tag='hsq')
                        nc.vector.tensor_tensor(hsq[:], h0a[:, 512 * g:512 * g + 512],
                                                h0a[:, 512 * g:512 * g + 512], AO.mult)
                        for j in range(4):
                            m = 4 * g + j
                            nc.gpsimd.tensor_scalar(out=E0[m], in0=hsq[:, 128 * j:128 * j + 128],
                                                    scalar1=c0n[:, m:m + 1],
                                                    scalar2=c0[:, m:m + 1],
                                                    op0=AO.mult, op1=AO.add)

                    # ---- layer 1 (+ E-path combine)
                    h1a = hpool.tile([128, DH], bf16, name='h1a', tag='h1a')
                    E1a = hpool.tile([128, DH], bf16, name='E1a', tag='E1a')
                    h1 = [h1a[:, 128 * m:128 * m + 128] for m in range(NK)]
                    E1 = [E1a[:, 128 * m:128 * m + 128] for m in range(NK)]
                    for g in range(4):
                        ps = pp.tile([128, 512], f32, name='zmain', tag='zmain')
                        for j in range(4):
                            m = 4 * g + j
                            sl = ps[:, 128 * j:128 * j + 128]
                            for k in range(m):
                                off = 128 * (m - k - 1)
                                nc.tensor.matmul(
                                    sl, w1chunks[k][:, off:off + 128],
                                    h0[k], start=(k == 0), stop=False)
                            nc.tensor.matmul(sl, w1dc[:, 128 * m:128 * m + 128],
                                             h0[m], start=(m == 0), stop=True)
                        psE = pp1.tile([128, 512], f32, name='epath', tag='epath')
                        for j in range(4):
                            m = 4 * g + j
                            nc.tensor.matmul(psE[:, 128 * j:128 * j + 128],
                                             w1de[:, 128 * m:128 * m + 128], E0[m],
                                             start=True, stop=True)
                        for j in range(4):
                            m = 4 * g + j
                            sl = ps[:, 128 * j:128 * j + 128]
                            nc.scalar.activation(h1[m], sl, AF.Tanh,
                                                 bias=b1[:, m:m + 1], scale=s1[:, m:m + 1])
                        hsq = spool.tile([128, 512], bf16, name='hsq', tag='hsq')
                        nc.vector.tensor_tensor(hsq[:], h1a[:, 512 * g:512 * g + 512],
                                                h1a[:, 512 * g:512 * g + 512], AO.mult)
                        etsw = spool.tile([128, 512], bf16, name='etsw', tag='etsw')
                        for j in range(4):
                            m = 4 * g + j
                            nc.gpsimd.tensor_scalar(out=etsw[:, 128 * j:128 * j + 128],
                                                    in0=hsq[:, 128 * j:128 * j + 128],
                                                    scalar1=s1n[:, m:m + 1],
                                                    scalar2=s1[:, m:m + 1],
                                                    op0=AO.mult, op1=AO.add)
                        nc.vector.tensor_tensor(E1a[:, 512 * g:512 * g + 512],
                                                psE[:, :], etsw[:], AO.mult)

                    # ---- layer 2
                    psz2 = ppw.tile([DIM, BC], f32, name='zsmall', tag='zsmall')
                    for k in range(NK):
                        nc.tensor.matmul(psz2[:], w2c[:, k * DIM:(k + 1) * DIM], h1[k],
                                         start=(k == 0), stop=(k == NK - 1))
                    z2 = spool.tile([DIM, BC], f32, name='z2s', tag='z2s')
                    nc.scalar.activation(z2[:], psz2[:], AF.Identity,
                                         bias=b2[:, 0:1], scale=s2[:, 0:1])
                    psE2 = pp1.tile([DIM, BC], f32, name='epath2', tag='epath')
                    for k in range(NK):
                        nc.tensor.matmul(psE2[:], w2e[:, k * DIM:(k + 1) * DIM],
                                         E1[k], start=(k == 0), stop=(k == NK - 1))
                    nc.vector.tensor_scalar(out=E2s[f][:], in0=psE2[:],
                                            scalar1=s2[:, 0:1], scalar2=None,
                                            op0=AO.mult)
                    if f < 2:
                        nc.vector.tensor_scalar(out=E2s[f][:], in0=E2s[f][:],
                                                scalar1=egs[f][:], scalar2=1.0,
                                                op0=AO.mult, op1=AO.add)

                    # ---- gate mix / flip or final logp term
                    if f < 2:
                        th = spool.tile([DIM, 1], f32, name='th', tag='th')
                        nc.scalar.activation(th[:], gc[:], AF.Tanh, scale=0.5)
                        sg = spool.tile([DIM, 1], f32, name='sg', tag='sg')
                        nc.vector.tensor_scalar(out=sg[:], in0=th[:], scalar1=0.5,
                                                scalar2=0.5, op0=AO.mult, op1=AO.add)
                        # xmix = sg*(z2 - x) + x; the next flow consumes xmix
                        # directly (flip folded into its w0s row order); the
                        # flipped copy is only needed for the next gate mix,
                        # built off the critical path
                        td = spool.tile([DIM, BC], f32, name='td', tag='td')
                        nc.vector.tensor_tensor(td[:], z2[:], xgate, AO.subtract)
                        nc.vector.tensor_scalar(out=td[:], in0=td[:], scalar1=sg[:],
                                                scalar2=None, op0=AO.mult)
                        x2 = cpool.tile([128, BC], bf16, name='x2', tag='x2',
                                        bufs=2)
                        nc.vector.tensor_tensor(x2[:DIM, :], td[:], xgate, AO.add)
                        nc.scalar.activation(x2[DIM:, :], x2[:DIM, :], AF.Copy)
                        psf = ppw.tile([DIM, BC], f32, name='zsmall', tag='zsmall')
                        nc.tensor.matmul(psf[:], flip64[:], x2[:DIM, :],
                                         start=True, stop=True)
                        xflip = spool.tile([DIM, BC], bf16, name='xflip', tag='xflip')
                        nc.scalar.activation(xflip[:], psf[:], AF.Copy)
                        xgate = xflip[:]
                    else:
                        sqx = spool.tile([DIM, BC], f32, name='sqx', tag='sqx')
                        nc.gpsimd.tensor_tensor(sqx[:], z2[:], z2[:], AO.mult)
                        nc.vector.tensor_scalar(out=logp[:], in0=sqx[:], scalar1=-0.5,
                                                scalar2=-0.5 * LOG_2PI, op0=AO.mult,
                                                op1=AO.add)

              # ---- ldj tail: l1p ln first (absorbs the act-table load in
              # an idle slot), then one wide ln; PE accumulates all feature
              # sums directly into the (1, BC) output PSUM
              l1p = spool.tile([DIM, 2], f32, name='l1p', tag='l1p')
              for f in range(2):
                    nc.vector.tensor_scalar(out=l1p[:, f:f + 1], in0=egs[f][:],
                                            scalar1=1.0, scalar2=None, op0=AO.add)
              nc.scalar.activation(l1p[:], l1p[:], AF.Ln)
              negp = spool.tile([DIM, 1], f32, name='negp', tag='negp')
              nc.vector.tensor_tensor(negp[:], l1p[:, 0:1], l1p[:, 1:2], AO.add)
              nc.vector.tensor_scalar(out=negp[:], in0=negp[:], scalar1=-1.0,
                                      scalar2=None, op0=AO.mult)
              lfall = spool.tile([DIM, 3 * BC], bf16, name='lfall', tag='lfall')
              nc.scalar.activation(lfall[:], E2all[:], AF.Ln)
              psum_out = ppw.tile([1, BC], f32, name='zsmall', tag='zsmall')
              nc.tensor.matmul(psum_out[:], ones64b[:], logp[:], start=True, stop=False)
              for f in range(3):
                    nc.tensor.matmul(psum_out[:], ones64b[:],
                                     lfall[:, f * BC:(f + 1) * BC],
                                     start=False, stop=False)
              nc.tensor.matmul(psum_out[:], negp[:], ones1bc[:],
                               start=False, stop=True)
              outs = spool.tile([1, BC], f32, name='outs', tag='outs')
              nc.vector.tensor_copy(outs[:], psum_out[:])
              nc.sync.dma_start(out=out[:], in_=outs[:])

    _split_sync_waits(nc)
    return nc


def _scol(nc, spool, AO, AF, wsnp, edw, shape, tag, iters=None, extra=None):
    """s = exp(dw) * rsqrt(wsn) via DVE bit-trick + Newton (no Ln/Exp)."""
    if iters is None:
        iters = RSQRT_ITERS
    import concourse.mybir as mybir
    f32 = mybir.dt.float32
    u32 = mybir.dt.uint32
    w = spool.tile(list(shape), f32, name=tag + 'w')
    if extra is None:
        nc.vector.tensor_copy(w[:], wsnp[:])
    else:
        # only one PSUM operand per DVE op: copy extra (all cols valid),
        # then add the main region (col 0 has no off-diag writes)
        nc.vector.tensor_copy(w[:], extra[:])
        nc.vector.tensor_tensor(w[:, 1:], w[:, 1:], wsnp[:, 1:], AO.add)
    y = spool.tile(list(shape), u32, name=tag + 'y')
    nc.vector.tensor_scalar(out=y[:], in0=w[:].bitcast(u32), scalar1=1,
                            scalar2=None, op0=AO.logical_shift_right)
    nc.vector.tensor_scalar(out=y[:], in0=y[:], scalar1=-1, scalar2=0x5f3759df,
                            op0=AO.mult, op1=AO.add)
    yf = y[:].bitcast(f32)
    a = spool.tile(list(shape), f32, name=tag + 'a')
    for _ in range(iters):
        nc.vector.tensor_tensor(a[:], yf, yf, AO.mult)
        nc.vector.tensor_tensor(a[:], a[:], w[:], AO.mult)
        nc.vector.tensor_scalar(out=a[:], in0=a[:], scalar1=-0.5, scalar2=1.5,
                                op0=AO.mult, op1=AO.add)
        nc.vector.tensor_tensor(yf, yf, a[:], AO.mult)
    s = spool.tile(list(shape), f32, name=tag + 's')
    nc.vector.tensor_tensor(s[:], yf, edw[:], AO.mult)
    return s


RSQRT_ITERS = 2


# ------------------------------------------------------------------ runner
def _make_runner(nc, n_cores):
    import jax
    from jax.sharding import Mesh, PartitionSpec
    from jax.experimental.shard_map import shard_map
    import concourse.mybir as mybir
    from concourse.bass2jax import (_bass_exec_p, partition_id_tensor,
                                    install_neuronx_cc_hook)
    install_neuronx_cc_hook()
    partition_name = nc.partition_id_tensor.name if nc.partition_id_tensor else None
    in_names, out_names, out_avals = [], [], []
    for alloc in nc.m.functions[0].allocations:
        if not isinstance(alloc, mybir.MemoryLocationSet):
            continue
        name = alloc.memorylocations[0].name
        if alloc.kind == "ExternalInput":
            if name != partition_name:
                in_names.append(name)
        elif alloc.kind == "ExternalOutput":
            out_names.append(name)
            out_avals.append(jax.core.ShapedArray(
                tuple(alloc.tensor_shape), mybir.dt.np(alloc.dtype)))
    n_params = len(in_names)
    all_names = in_names + out_names + ([partition_name] if partition_name else [])

    def _body(*args):
        operands = list(args)
        if partition_name is not None:
            operands.append(partition_id_tensor())
        outs = _bass_exec_p.bind(
            *operands, out_avals=tuple(out_avals), in_names=tuple(all_names),
            out_names=tuple(out_names), lowering_input_output_aliases=(),
            sim_require_finite=False, sim_require_nnan=False, nc=nc)
        return tuple(outs)

    devices = jax.devices()[:n_cores]
    mesh = Mesh(np.asarray(devices), ("core",))
    n_outs = len(out_names)
    in_specs = (PartitionSpec("core"),) * (n_params + n_outs)
    out_specs = (PartitionSpec("core"),) * n_outs
    fn = jax.jit(shard_map(_body, mesh=mesh, in_specs=in_specs,
                           out_specs=out_specs, check_rep=False),
                 keep_unused=True)
    return fn, in_names, out_names, out_avals


def _get_runner():
    key = ('runner', MAIN_DT)
    if key not in _CACHE:
        import sys, os
        d = os.path.dirname(os.path.abspath(__file__))
        if d not in sys.path:
            sys.path.insert(0, d)
        nc = _build(MAIN_DT)
        _CACHE[key] = _make_runner(nc, NCORES)
    return _CACHE[key]


def kernel(**inputs):
    fl = _host_prep(inputs)
    x = np.asarray(inputs['x'])
    fn, in_names, out_names, out_avals = _get_runner()
    in_maps = []
    for c in range(NCORES):
        m = dict(fl)
        m['xT'] = np.ascontiguousarray(x[c * BC:(c + 1) * BC, :].T).astype(BF16)
        in_maps.append(m)
    concat_in = [np.concatenate([np.asarray(m[name]) for m in in_maps], axis=0)
                 for name in in_names]
    concat_zeros = [np.zeros((NCORES * a.shape[0], *a.shape[1:]), a.dtype)
                    for a in out_avals]
    outs = fn(*concat_in, *concat_zeros)
    o = np.asarray(outs[0]).reshape(NCORES, BC)
    return o.reshape(B).astype(np.float32)
